# revision 19
# baseline (speedup 1.0000x reference)
"""GroupSupConLoss on 8 Trainium2 NeuronCores — symmetric (triangle) version.

Device computes Z[i] = sum_{j != i} exp(sim[i,j]) exploiting sim's symmetry:
each exp'd element feeds TWO Z entries (its row via ACT accum row-sums, its
column via fp8 DoubleRow ones-matmul column-sums), so each core computes only
~9/16 of the columns the naive row-block split needs.

Coverage (rotated coords; col j of core c is global (c*1024+j) % 8192).
Row tiles rt 0-7 are the core's own 1024 rows; per-chunk column extents:
  C0 Da   : rts 0-3 x cols    0:1024  rowsums; colsum on cols 512:1024
  C1 DbB3 : rts 4-7 x cols  512:1024 | 4096:4608   rowsums; colsum on B3 part
  C2 B1   : rts 0-7 x cols 1024:2560  rowsums + colsums
  C3 B2   : rts 0-7 x cols 2560:4096  rowsums + colsums
  C4 X    : distance-8 512x512 quadrant (per-core stationary wxpl / moving
            exin): cores 0-3 rows 0:512 x global strip 2c+8;
                   cores 4-7 rows 512:1024 x global strip 2c-7
The self-diagonal exp(sim[i,i]) is NOT masked on device; the host subtracts
exp(||e_i||^2_fp8 / tau) computed exactly from the quantized embeddings.
Host maps col-sum vectors back to global rows, assembles Z, then does the
O(B*D) positives math in float64.

GEMM is fp8e4 (embeddings scaled by 32), DoubleRow perf mode (K=256 per
instruction), with bank-inner stationary reuse to amortize PE weight loads
(which are not hidden on real TRN2 and dominate per-matmul cost).
"""

import numpy as np
import ml_dtypes

import concourse.bacc as bacc
import concourse.mybir as mybir
from concourse.tile import TileContext

B = 8192           # batch
D = 1024           # embed dim
NCORES = 8
RPC = B // NCORES  # rows per core = 1024
NK = D // 128      # 8 contraction chunks
NKP = NK // 2      # 4 DoubleRow k-pairs
NRT = RPC // 128   # 8 row tiles per core
WCOLS = 1024       # resident own-block cols
TAU = 0.1
FP8_SCALE = 32.0
ISCALE = 1.0 / (TAU * FP8_SCALE * FP8_SCALE)

DRMODE = mybir.MatmulPerfMode.DoubleRow

_NC_CACHE = {}


def _build_nc(
    reps: int = 1,
    fori: bool = False,
    no_csum: bool = False,
    ex_bufs: int = 3,
    x_early: bool = True,
):
    nc = bacc.Bacc(None, target_bir_lowering=False)
    f8 = mybir.dt.float8e4
    # rotated E^T cols 0:4608 (all this core reads)
    etrot = nc.declare_dram_parameter("etrot", [D, 4608], f8, isOutput=False)
    # X-quadrant stationary (own 512-row half) and moving operand
    wxpl = nc.declare_dram_parameter("wxpl", [D, 512], f8, isOutput=False)
    exin = nc.declare_dram_parameter("exin", [D, 512], f8, isOutput=False)
    onesd = nc.declare_dram_parameter("onesd", [128, 256], f8, isOutput=False)
    zrow = nc.declare_dram_parameter("zrow", [128, NRT], mybir.dt.float32, isOutput=True)
    zx = nc.declare_dram_parameter("zx", [128, 4], mybir.dt.float32, isOutput=True)
    zcol = nc.declare_dram_parameter("zcol", [1, 9 * 512], mybir.dt.float32, isOutput=True)

    et3 = etrot.rearrange("(nk p) c -> p nk c", p=128)
    wx3 = wxpl.rearrange("(nk p) c -> p nk c", p=128)
    ex3 = exin.rearrange("(nk p) c -> p nk c", p=128)

    with TileContext(nc) as tc:
        with (
            tc.tile_pool(name="singles", bufs=1) as singles,
            tc.tile_pool(name="rhsp", bufs=2) as rhsp,
            tc.tile_pool(name="psump", bufs=2, space="PSUM") as psump,
            tc.tile_pool(name="cpsum", bufs=2, space="PSUM") as cpsum,
            tc.tile_pool(name="expp", bufs=ex_bufs) as expp,
        ):
            W = singles.tile([128, NK, WCOLS], f8, name="W")
            for k in range(NK):
                nc.sync.dma_start(out=W[:, k : k + 1, :], in_=et3[:, k : k + 1, 0:WCOLS])
            wx_sb = singles.tile([128, NK, 512], f8, name="wx_sb")
            nc.sync.dma_start(out=wx_sb[:, :, :], in_=wx3[:, :, :])
            exin_sb = singles.tile([128, NK, 512], f8, name="exin_sb")
            nc.sync.dma_start(out=exin_sb[:, :, :], in_=ex3[:, :, :])
            ones_sb = singles.tile([128, 2, 128], f8, name="ones_sb")
            nc.sync.dma_start(
                out=ones_sb[:, :, :],
                in_=onesd.rearrange("p (two m) -> p two m", two=2)[:, :, :],
            )

            acc = singles.tile([128, NRT, 3], mybir.dt.float32, name="acc")
            accx = singles.tile([128, 4], mybir.dt.float32, name="accx")
            zt = singles.tile([128, NRT], mybir.dt.float32, name="zt")
            zc_sb = singles.tile([1, 9 * 512], mybir.dt.float32, name="zc_sb")
            if no_csum:
                nc.vector.memset(zc_sb[:, :], 0.0)

            # chunk descriptors:
            # (name, rts, nunits, moving_fn, stationary_fn, slot, csum_units)
            # moving_fn(kp, u) -> AP [128, 2, 512]; csum_units: list of
            # (unit_idx_in_chunk, zcol_unit_idx)
            def _body(rep):
                # --- C0 "Da": rts 0-3, cols 0:1024 ---
                # --- C1 "DbB3": rts 4-7, cols 512:1024 | 4096:4608 ---
                rhs_b3 = rhsp.tile([128, NK, 512], f8, name=f"rhsb3_{rep}", tag="rhs3")
                nc.sync.dma_start(out=rhs_b3[:, :, :], in_=et3[:, :, 4096:4608])
                # --- C2/C3 B-groups 1536 cols each ---
                rhs_b1 = rhsp.tile([128, NK, 1536], f8, name=f"rhsb1_{rep}", tag="rhs")
                nc.sync.dma_start(out=rhs_b1[:, :, :], in_=et3[:, :, 1024:2560])
                rhs_b2 = rhsp.tile([128, NK, 1536], f8, name=f"rhsb2_{rep}", tag="rhs")
                nc.sync.dma_start(out=rhs_b2[:, :, :], in_=et3[:, :, 2560:4096])

                chunks = [
                    # name, rt range, cols, moving aps per (kp, unit), slot,
                    # csum: list of (unit, zcol_slot)
                    ("Da", range(0, 4), 1024,
                     lambda kp, u: W[:, 2 * kp : 2 * kp + 2, u * 512 : u * 512 + 512],
                     None, 0, [(1, 0)]),
                    ("DbB3", range(4, 8), 1024,
                     lambda kp, u: (
                         W[:, 2 * kp : 2 * kp + 2, 512:1024] if u == 0
                         else rhs_b3[:, 2 * kp : 2 * kp + 2, :]
                     ),
                     None, 0, [(1, 1)]),
                    ("B1", range(0, 8), 1536,
                     lambda kp, u: rhs_b1[:, 2 * kp : 2 * kp + 2, u * 512 : u * 512 + 512],
                     None, 1, [(0, 2), (1, 3), (2, 4)]),
                    ("B2", range(0, 8), 1536,
                     lambda kp, u: rhs_b2[:, 2 * kp : 2 * kp + 2, u * 512 : u * 512 + 512],
                     None, 2, [(0, 5), (1, 6), (2, 7)]),
                    ("X", range(0, 4), 512,
                     lambda kp, u: exin_sb[:, 2 * kp : 2 * kp + 2, :],
                     wx_sb, None, [(0, 8)]),
                ]
                if x_early:
                    chunks = [chunks[0], chunks[4], chunks[1], chunks[2], chunks[3]]

                for name, rts, cols, moving, statsrc, slot, csums in chunks:
                    nunits = cols // 512
                    nrts = len(list(rts))
                    ex3d = expp.tile(
                        [128, nrts, cols], f8, name=f"ex_{rep}_{name}", tag="ex"
                    )
                    for ri, rt in enumerate(rts):
                        ps = psump.tile(
                            [128, cols], mybir.dt.float32,
                            name=f"ps_{rep}_{name}_{rt}", tag="ps",
                        )
                        for kp in range(NKP):
                            if statsrc is None:
                                lhsT = W[:, 2 * kp : 2 * kp + 2, rt * 128 : (rt + 1) * 128]
                            else:
                                lhsT = statsrc[:, 2 * kp : 2 * kp + 2, ri * 128 : (ri + 1) * 128]
                            for u in range(nunits):
                                nc.tensor.matmul(
                                    ps[:, u * 512 : (u + 1) * 512],
                                    lhsT,
                                    moving(kp, u),
                                    start=(kp == 0),
                                    stop=(kp == NKP - 1),
                                    perf_mode=DRMODE,
                                    skip_group_check=True,
                                )
                        if name == "X":
                            acc_slot = accx[:, ri : ri + 1]
                        else:
                            acc_slot = acc[:, rt, slot : slot + 1]
                        nc.scalar.activation(
                            out=ex3d[:, ri, :],
                            in_=ps,
                            func=mybir.ActivationFunctionType.Exp,
                            scale=ISCALE,
                            accum_out=acc_slot,
                        )
                    # column sums for the marked units: fp8 DoubleRow
                    # ones-matmuls over rt pairs
                    for u, zslot in (() if no_csum else csums):
                        pscol = cpsum.tile(
                            [128, 512], mybir.dt.float32,
                            name=f"pscol_{rep}_{name}_{u}", tag="pscol",
                        )
                        npr = nrts // 2
                        for pr in range(npr):
                            nc.tensor.matmul(
                                pscol,
                                ones_sb[:, :, :],
                                ex3d[:, 2 * pr : 2 * pr + 2, u * 512 : (u + 1) * 512],
                                start=(pr == 0),
                                stop=(pr == npr - 1),
                                perf_mode=DRMODE,
                                skip_group_check=True,
                            )
                        nc.vector.tensor_copy(
                            zc_sb[:, zslot * 512 : (zslot + 1) * 512], pscol[0:1, :]
                        )

            if fori and reps > 1:
                with tc.For_i(0, reps):
                    _body(0)
            else:
                for rep in range(reps):
                    _body(rep)

            nc.vector.reduce_sum(zt[:, :], acc[:, :, :], axis=mybir.AxisListType.X)
            nc.sync.dma_start(out=zrow[:, :], in_=zt)
            nc.sync.dma_start(out=zx[:, :], in_=accx)
            nc.sync.dma_start(out=zcol[:, :], in_=zc_sb)
    nc.finalize()
    return nc


def _get_nc():
    if "nc" not in _NC_CACHE:
        _NC_CACHE["nc"] = _build_nc()
    return _NC_CACHE["nc"]


def _make_runner(nc=None, key="runner"):
    """Build a cached jitted SPMD executor for the bass program."""
    if key in _NC_CACHE:
        return _NC_CACHE[key]

    import jax
    import concourse.mybir as mybir_
    from concourse import bass2jax
    from concourse.bass2jax import _bass_exec_p, partition_id_tensor
    from jax.sharding import Mesh, PartitionSpec
    from jax.experimental.shard_map import shard_map

    if nc is None:
        nc = _get_nc()
    bass2jax.install_neuronx_cc_hook()

    partition_name = nc.partition_id_tensor.name if nc.partition_id_tensor else None
    in_names, out_names, out_avals, zero_outs = [], [], [], []
    for alloc in nc.m.functions[0].allocations:
        if not isinstance(alloc, mybir_.MemoryLocationSet):
            continue
        name = alloc.memorylocations[0].name
        if alloc.kind == "ExternalInput":
            if name != partition_name:
                in_names.append(name)
        elif alloc.kind == "ExternalOutput":
            shape = tuple(alloc.tensor_shape)
            dtype = mybir_.dt.np(alloc.dtype)
            out_names.append(name)
            out_avals.append(jax.core.ShapedArray(shape, dtype))
            zero_outs.append(np.zeros(shape, dtype))
    n_params = len(in_names)
    all_in_names = list(in_names) + list(out_names)
    if partition_name is not None:
        all_in_names.append(partition_name)
    donate = tuple(range(n_params, n_params + len(out_avals)))

    def _bodyfn(*args):
        operands = list(args)
        if partition_name is not None:
            operands.append(partition_id_tensor())
        outs = _bass_exec_p.bind(
            *operands,
            out_avals=tuple(out_avals),
            in_names=tuple(all_in_names),
            out_names=tuple(out_names),
            lowering_input_output_aliases=(),
            sim_require_finite=True,
            sim_require_nnan=True,
            nc=nc,
        )
        return tuple(outs)

    devices = jax.devices()[:NCORES]
    mesh = Mesh(np.asarray(devices), ("core",))
    spec = PartitionSpec("core")
    sharded = jax.jit(
        shard_map(
            _bodyfn,
            mesh=mesh,
            in_specs=(spec,) * (n_params + len(out_avals)),
            out_specs=(spec,) * len(out_names),
            check_rep=False,
        ),
        donate_argnums=donate,
        keep_unused=True,
    )

    def run(in_maps, staged=None):
        if staged is None:
            concat_in = [
                np.concatenate([np.asarray(m[name]) for m in in_maps], axis=0)
                for name in in_names
            ]
        else:
            concat_in = staged
        concat_zeros = [
            np.zeros((NCORES * z.shape[0], *z.shape[1:]), z.dtype) for z in zero_outs
        ]
        out_arrs = sharded(*concat_in, *concat_zeros)
        return [
            {
                name: np.asarray(out_arrs[i]).reshape(NCORES, *out_avals[i].shape)[c]
                for i, name in enumerate(out_names)
            }
            for c in range(NCORES)
        ]

    run.in_names = in_names
    run.mesh = mesh
    run.spec = spec
    run.sharded = sharded
    run.zero_outs = zero_outs
    _NC_CACHE[key] = run
    return run


def _quantize(embeddings_f32: np.ndarray):
    """Returns (et8, e8f): [D, B] fp8 of E^T*scale, and [B, D] float32 of the
    dequantized values (for the host diag correction)."""
    et8 = np.ascontiguousarray(embeddings_f32.T * FP8_SCALE).astype(
        ml_dtypes.float8_e4m3
    )
    return et8


def _make_in_maps(embeddings_f32: np.ndarray):
    et = _quantize(embeddings_f32)  # [D, B] fp8
    ones2 = np.ones((128, 256), dtype=ml_dtypes.float8_e4m3)

    in_maps = []
    for c in range(NCORES):
        etrot_full = np.roll(et, -c * RPC, axis=1)
        etrot = np.ascontiguousarray(etrot_full[:, :4608])
        if c < 4:
            exin = etrot_full[:, 4096:4608]   # global strip 2c+8
            wxpl = etrot_full[:, 0:512]       # own rows 0:512
        else:
            exin = etrot_full[:, 4608:5120]   # global strip 2c-7
            wxpl = etrot_full[:, 512:1024]    # own rows 512:1024
        in_maps.append(
            {
                "etrot": etrot,
                "wxpl": np.ascontiguousarray(wxpl),
                "exin": np.ascontiguousarray(exin),
                "onesd": ones2,
            }
        )
    return in_maps


def _assemble_Z(results):
    Z = np.zeros(B, dtype=np.float64)
    for c in range(NCORES):
        r0 = c * RPC
        zr = np.asarray(results[c]["zrow"], np.float64)  # [128, 8]
        Z[r0 : r0 + RPC] += zr.T.reshape(-1)
        zxv = np.asarray(results[c]["zx"], np.float64)  # [128, 4]
        xoff = 0 if c < 4 else 512
        Z[r0 + xoff : r0 + xoff + 512] += zxv.T.reshape(-1)
        zc = np.asarray(results[c]["zcol"], np.float64).reshape(-1)  # [9*512]
        # units 0..8 -> rotated col ranges
        unit_cols = [512, 4096, 1024, 1536, 2048, 2560, 3072, 3584, None]
        for unit in range(8):
            j0 = unit_cols[unit]
            gcols = (r0 + j0 + np.arange(512)) % B
            Z[gcols] += zc[unit * 512 : (unit + 1) * 512]
        # unit 8: X cols
        xc0 = ((2 * c + 8) * 512) % B if c < 4 else ((2 * c - 7) * 512) % B
        Z[xc0 : xc0 + 512] += zc[8 * 512 : 9 * 512]
    return Z


def _device_Z(embeddings_f32: np.ndarray):
    run = _make_runner()
    results = run(_make_in_maps(embeddings_f32))
    Z = _assemble_Z(results)
    # subtract the unmasked diagonal: exp(||e_i||^2_fp8 * ISCALE)
    et = _quantize(embeddings_f32).astype(np.float64)  # [D, B]
    s_ii = np.einsum("di,di->i", et, et)
    Z -= np.exp(s_ii * ISCALE)
    return Z


def kernel(embeddings: np.ndarray, labels: np.ndarray) -> np.ndarray:
    E = np.asarray(embeddings, dtype=np.float32)
    labels = np.asarray(labels)

    Z = _device_Z(E)

    # Host epilogue in float64 (O(B*D) work).
    Ef = E.astype(np.float64)
    lse = np.log(Z)

    nclass = int(labels.max()) + 1
    counts = np.bincount(labels, minlength=nclass)
    num_pos = counts[labels] - 1
    G = np.zeros((nclass, D), dtype=np.float64)
    np.add.at(G, labels, Ef)
    sum_pos = (
        np.einsum("ij,ij->i", Ef, G[labels]) - np.einsum("ij,ij->i", Ef, Ef)
    ) / TAU
    mean_pos = sum_pos / np.maximum(num_pos, 1)
    has_pos = num_pos > 0
    loss_i = lse - mean_pos
    loss = np.sum(np.where(has_pos, loss_i, 0.0)) / max(int(has_pos.sum()), 1)
    return np.float32(loss)


# revision 22
# speedup vs baseline: 1.0491x; 1.0491x over previous
"""GroupSupConLoss on 8 Trainium2 NeuronCores — symmetric (triangle) version.

Device computes Z[i] = sum_{j != i} exp(sim[i,j]) exploiting sim's symmetry:
each exp'd element feeds TWO Z entries (its row via ACT accum row-sums, its
column via fp8 DoubleRow ones-matmul column-sums), so each core computes only
~9/16 of the columns the naive row-block split needs.

Coverage (rotated coords; col j of core c is global (c*1024+j) % 8192).
Row tiles rt 0-7 are the core's own 1024 rows; per-chunk column extents:
  C0 Da   : rts 0-3 x cols    0:1024  rowsums; colsum on cols 512:1024
  C1 DbB3 : rts 4-7 x cols  512:1024 | 4096:4608   rowsums; colsum on B3 part
  C2 B1   : rts 0-7 x cols 1024:2560  rowsums + colsums
  C3 B2   : rts 0-7 x cols 2560:4096  rowsums + colsums
  C4 X    : distance-8 512x512 quadrant (per-core stationary wxpl / moving
            exin): cores 0-3 rows 0:512 x global strip 2c+8;
                   cores 4-7 rows 512:1024 x global strip 2c-7
The self-diagonal exp(sim[i,i]) is NOT masked on device; the host subtracts
exp(||e_i||^2_fp8 / tau) computed exactly from the quantized embeddings.
Host maps col-sum vectors back to global rows, assembles Z, then does the
O(B*D) positives math in float64.

GEMM is fp8e4 (embeddings scaled by 32), DoubleRow perf mode (K=256 per
instruction), with bank-inner stationary reuse to amortize PE weight loads
(which are not hidden on real TRN2 and dominate per-matmul cost).
"""

import numpy as np
import ml_dtypes

import concourse.bacc as bacc
import concourse.mybir as mybir
from concourse.tile import TileContext

B = 8192           # batch
D = 1024           # embed dim
NCORES = 8
RPC = B // NCORES  # rows per core = 1024
NK = D // 128      # 8 contraction chunks
NKP = NK // 2      # 4 DoubleRow k-pairs
NRT = RPC // 128   # 8 row tiles per core
WCOLS = 1024       # resident own-block cols
TAU = 0.1
FP8_SCALE = 32.0
ISCALE = 1.0 / (TAU * FP8_SCALE * FP8_SCALE)

DRMODE = mybir.MatmulPerfMode.DoubleRow

_NC_CACHE = {}


def _dedup_ldweights(nc):
    """Remove back-to-back InstLdweights that reload the identical stationary
    operand (same tensor/offset/access-pattern/perf-mode). The PE weight array
    persists across matmuls, and walrus pairs each non-self-loading matmul
    with the most recent ldweights, so consecutive duplicates are pure
    overhead (~107ns each for fp8 DoubleRow loads). Loads carrying semaphore
    waits or updates are kept (bacc moved matmul waits onto them)."""
    removed = 0
    for fn in nc.m.functions:
        for bb in fn.blocks:
            il = bb.instructions
            last_key = None
            i = 0
            while i < len(il):
                inst = il[i]
                if type(inst).__name__ == "InstLdweights":
                    ap = inst.ins[0]
                    key = (
                        getattr(ap, "offset", None),
                        str(getattr(ap, "ap", None)),
                        str(getattr(ap, "memref", None)),
                        str(getattr(inst, "perf_mode", None)),
                    )
                    si = inst.sync_info
                    clean = si is None or (not si.on_wait and not si.on_update)
                    if key == last_key and clean:
                        del il[i]
                        removed += 1
                        continue
                    last_key = key
                i += 1
    return removed


def _build_nc(
    reps: int = 1,
    fori: bool = False,
    no_csum: bool = False,
    ex_bufs: int = 3,
    x_early: bool = True,
    dedup_lw: bool = True,
):
    nc = bacc.Bacc(None, target_bir_lowering=False)
    f8 = mybir.dt.float8e4
    # rotated E^T cols 0:4608 (all this core reads)
    etrot = nc.declare_dram_parameter("etrot", [D, 4608], f8, isOutput=False)
    # X-quadrant stationary (own 512-row half) and moving operand
    wxpl = nc.declare_dram_parameter("wxpl", [D, 512], f8, isOutput=False)
    exin = nc.declare_dram_parameter("exin", [D, 512], f8, isOutput=False)
    onesd = nc.declare_dram_parameter("onesd", [128, 256], f8, isOutput=False)
    zrow = nc.declare_dram_parameter("zrow", [128, NRT], mybir.dt.float32, isOutput=True)
    zx = nc.declare_dram_parameter("zx", [128, 4], mybir.dt.float32, isOutput=True)
    zcol = nc.declare_dram_parameter("zcol", [1, 9 * 512], mybir.dt.float32, isOutput=True)

    et3 = etrot.rearrange("(nk p) c -> p nk c", p=128)
    wx3 = wxpl.rearrange("(nk p) c -> p nk c", p=128)
    ex3 = exin.rearrange("(nk p) c -> p nk c", p=128)

    with TileContext(nc) as tc:
        with (
            tc.tile_pool(name="singles", bufs=1) as singles,
            tc.tile_pool(name="rhsp", bufs=2) as rhsp,
            tc.tile_pool(name="psump", bufs=2, space="PSUM") as psump,
            tc.tile_pool(name="cpsum", bufs=2, space="PSUM") as cpsum,
            tc.tile_pool(name="expp", bufs=ex_bufs) as expp,
        ):
            W = singles.tile([128, NK, WCOLS], f8, name="W")
            for k in range(NK):
                nc.sync.dma_start(out=W[:, k : k + 1, :], in_=et3[:, k : k + 1, 0:WCOLS])
            wx_sb = singles.tile([128, NK, 512], f8, name="wx_sb")
            nc.sync.dma_start(out=wx_sb[:, :, :], in_=wx3[:, :, :])
            exin_sb = singles.tile([128, NK, 512], f8, name="exin_sb")
            nc.sync.dma_start(out=exin_sb[:, :, :], in_=ex3[:, :, :])
            ones_sb = singles.tile([128, 2, 128], f8, name="ones_sb")
            nc.sync.dma_start(
                out=ones_sb[:, :, :],
                in_=onesd.rearrange("p (two m) -> p two m", two=2)[:, :, :],
            )

            acc = singles.tile([128, NRT, 3], mybir.dt.float32, name="acc")
            accx = singles.tile([128, 4], mybir.dt.float32, name="accx")
            zt = singles.tile([128, NRT], mybir.dt.float32, name="zt")
            zc_sb = singles.tile([1, 9 * 512], mybir.dt.float32, name="zc_sb")
            if no_csum:
                nc.vector.memset(zc_sb[:, :], 0.0)

            # chunk descriptors:
            # (name, rts, nunits, moving_fn, stationary_fn, slot, csum_units)
            # moving_fn(kp, u) -> AP [128, 2, 512]; csum_units: list of
            # (unit_idx_in_chunk, zcol_unit_idx)
            def _body(rep):
                # --- C0 "Da": rts 0-3, cols 0:1024 ---
                # --- C1 "DbB3": rts 4-7, cols 512:1024 | 4096:4608 ---
                rhs_b3 = rhsp.tile([128, NK, 512], f8, name=f"rhsb3_{rep}", tag="rhs3")
                nc.sync.dma_start(out=rhs_b3[:, :, :], in_=et3[:, :, 4096:4608])
                # --- C2/C3 B-groups 1536 cols each ---
                rhs_b1 = rhsp.tile([128, NK, 1536], f8, name=f"rhsb1_{rep}", tag="rhs")
                nc.sync.dma_start(out=rhs_b1[:, :, :], in_=et3[:, :, 1024:2560])
                rhs_b2 = rhsp.tile([128, NK, 1536], f8, name=f"rhsb2_{rep}", tag="rhs")
                nc.sync.dma_start(out=rhs_b2[:, :, :], in_=et3[:, :, 2560:4096])

                chunks = [
                    # name, rt range, cols, moving aps per (kp, unit), slot,
                    # csum: list of (unit, zcol_slot)
                    ("Da", range(0, 4), 1024,
                     lambda kp, u: W[:, 2 * kp : 2 * kp + 2, u * 512 : u * 512 + 512],
                     None, 0, [(1, 0)]),
                    ("DbB3", range(4, 8), 1024,
                     lambda kp, u: (
                         W[:, 2 * kp : 2 * kp + 2, 512:1024] if u == 0
                         else rhs_b3[:, 2 * kp : 2 * kp + 2, :]
                     ),
                     None, 0, [(1, 1)]),
                    ("B1", range(0, 8), 1536,
                     lambda kp, u: rhs_b1[:, 2 * kp : 2 * kp + 2, u * 512 : u * 512 + 512],
                     None, 1, [(0, 2), (1, 3), (2, 4)]),
                    ("B2", range(0, 8), 1536,
                     lambda kp, u: rhs_b2[:, 2 * kp : 2 * kp + 2, u * 512 : u * 512 + 512],
                     None, 2, [(0, 5), (1, 6), (2, 7)]),
                    ("X", range(0, 4), 512,
                     lambda kp, u: exin_sb[:, 2 * kp : 2 * kp + 2, :],
                     wx_sb, None, [(0, 8)]),
                ]
                if x_early:
                    chunks = [chunks[0], chunks[4], chunks[1], chunks[2], chunks[3]]

                for name, rts, cols, moving, statsrc, slot, csums in chunks:
                    nunits = cols // 512
                    nrts = len(list(rts))
                    ex3d = expp.tile(
                        [128, nrts, cols], f8, name=f"ex_{rep}_{name}", tag="ex"
                    )
                    for ri, rt in enumerate(rts):
                        ps = psump.tile(
                            [128, cols], mybir.dt.float32,
                            name=f"ps_{rep}_{name}_{rt}", tag="ps",
                        )
                        for kp in range(NKP):
                            if statsrc is None:
                                lhsT = W[:, 2 * kp : 2 * kp + 2, rt * 128 : (rt + 1) * 128]
                            else:
                                lhsT = statsrc[:, 2 * kp : 2 * kp + 2, ri * 128 : (ri + 1) * 128]
                            for u in range(nunits):
                                nc.tensor.matmul(
                                    ps[:, u * 512 : (u + 1) * 512],
                                    lhsT,
                                    moving(kp, u),
                                    start=(kp == 0),
                                    stop=(kp == NKP - 1),
                                    perf_mode=DRMODE,
                                    skip_group_check=True,
                                )
                        if name == "X":
                            acc_slot = accx[:, ri : ri + 1]
                        else:
                            acc_slot = acc[:, rt, slot : slot + 1]
                        nc.scalar.activation(
                            out=ex3d[:, ri, :],
                            in_=ps,
                            func=mybir.ActivationFunctionType.Exp,
                            scale=ISCALE,
                            accum_out=acc_slot,
                        )
                    # column sums for the marked units: fp8 DoubleRow
                    # ones-matmuls over rt pairs
                    for u, zslot in (() if no_csum else csums):
                        pscol = cpsum.tile(
                            [128, 512], mybir.dt.float32,
                            name=f"pscol_{rep}_{name}_{u}", tag="pscol",
                        )
                        npr = nrts // 2
                        for pr in range(npr):
                            nc.tensor.matmul(
                                pscol,
                                ones_sb[:, :, :],
                                ex3d[:, 2 * pr : 2 * pr + 2, u * 512 : (u + 1) * 512],
                                start=(pr == 0),
                                stop=(pr == npr - 1),
                                perf_mode=DRMODE,
                                skip_group_check=True,
                            )
                        nc.vector.tensor_copy(
                            zc_sb[:, zslot * 512 : (zslot + 1) * 512], pscol[0:1, :]
                        )

            if fori and reps > 1:
                with tc.For_i(0, reps):
                    _body(0)
            else:
                for rep in range(reps):
                    _body(rep)

            nc.vector.reduce_sum(zt[:, :], acc[:, :, :], axis=mybir.AxisListType.X)
            nc.sync.dma_start(out=zrow[:, :], in_=zt)
            nc.sync.dma_start(out=zx[:, :], in_=accx)
            nc.sync.dma_start(out=zcol[:, :], in_=zc_sb)
    nc.finalize()
    if dedup_lw:
        _dedup_ldweights(nc)
    return nc


def _get_nc():
    if "nc" not in _NC_CACHE:
        _NC_CACHE["nc"] = _build_nc()
    return _NC_CACHE["nc"]


def _make_runner(nc=None, key="runner"):
    """Build a cached jitted SPMD executor for the bass program."""
    if key in _NC_CACHE:
        return _NC_CACHE[key]

    import jax
    import concourse.mybir as mybir_
    from concourse import bass2jax
    from concourse.bass2jax import _bass_exec_p, partition_id_tensor
    from jax.sharding import Mesh, PartitionSpec
    from jax.experimental.shard_map import shard_map

    if nc is None:
        nc = _get_nc()
    bass2jax.install_neuronx_cc_hook()

    partition_name = nc.partition_id_tensor.name if nc.partition_id_tensor else None
    in_names, out_names, out_avals, zero_outs = [], [], [], []
    for alloc in nc.m.functions[0].allocations:
        if not isinstance(alloc, mybir_.MemoryLocationSet):
            continue
        name = alloc.memorylocations[0].name
        if alloc.kind == "ExternalInput":
            if name != partition_name:
                in_names.append(name)
        elif alloc.kind == "ExternalOutput":
            shape = tuple(alloc.tensor_shape)
            dtype = mybir_.dt.np(alloc.dtype)
            out_names.append(name)
            out_avals.append(jax.core.ShapedArray(shape, dtype))
            zero_outs.append(np.zeros(shape, dtype))
    n_params = len(in_names)
    all_in_names = list(in_names) + list(out_names)
    if partition_name is not None:
        all_in_names.append(partition_name)
    donate = tuple(range(n_params, n_params + len(out_avals)))

    def _bodyfn(*args):
        operands = list(args)
        if partition_name is not None:
            operands.append(partition_id_tensor())
        outs = _bass_exec_p.bind(
            *operands,
            out_avals=tuple(out_avals),
            in_names=tuple(all_in_names),
            out_names=tuple(out_names),
            lowering_input_output_aliases=(),
            sim_require_finite=True,
            sim_require_nnan=True,
            nc=nc,
        )
        return tuple(outs)

    devices = jax.devices()[:NCORES]
    mesh = Mesh(np.asarray(devices), ("core",))
    spec = PartitionSpec("core")
    sharded = jax.jit(
        shard_map(
            _bodyfn,
            mesh=mesh,
            in_specs=(spec,) * (n_params + len(out_avals)),
            out_specs=(spec,) * len(out_names),
            check_rep=False,
        ),
        donate_argnums=donate,
        keep_unused=True,
    )

    def run(in_maps, staged=None):
        if staged is None:
            concat_in = [
                np.concatenate([np.asarray(m[name]) for m in in_maps], axis=0)
                for name in in_names
            ]
        else:
            concat_in = staged
        concat_zeros = [
            np.zeros((NCORES * z.shape[0], *z.shape[1:]), z.dtype) for z in zero_outs
        ]
        out_arrs = sharded(*concat_in, *concat_zeros)
        return [
            {
                name: np.asarray(out_arrs[i]).reshape(NCORES, *out_avals[i].shape)[c]
                for i, name in enumerate(out_names)
            }
            for c in range(NCORES)
        ]

    run.in_names = in_names
    run.mesh = mesh
    run.spec = spec
    run.sharded = sharded
    run.zero_outs = zero_outs
    _NC_CACHE[key] = run
    return run


def _quantize(embeddings_f32: np.ndarray):
    """Returns (et8, e8f): [D, B] fp8 of E^T*scale, and [B, D] float32 of the
    dequantized values (for the host diag correction)."""
    et8 = np.ascontiguousarray(embeddings_f32.T * FP8_SCALE).astype(
        ml_dtypes.float8_e4m3
    )
    return et8


def _make_in_maps(embeddings_f32: np.ndarray):
    et = _quantize(embeddings_f32)  # [D, B] fp8
    ones2 = np.ones((128, 256), dtype=ml_dtypes.float8_e4m3)

    in_maps = []
    for c in range(NCORES):
        etrot_full = np.roll(et, -c * RPC, axis=1)
        etrot = np.ascontiguousarray(etrot_full[:, :4608])
        if c < 4:
            exin = etrot_full[:, 4096:4608]   # global strip 2c+8
            wxpl = etrot_full[:, 0:512]       # own rows 0:512
        else:
            exin = etrot_full[:, 4608:5120]   # global strip 2c-7
            wxpl = etrot_full[:, 512:1024]    # own rows 512:1024
        in_maps.append(
            {
                "etrot": etrot,
                "wxpl": np.ascontiguousarray(wxpl),
                "exin": np.ascontiguousarray(exin),
                "onesd": ones2,
            }
        )
    return in_maps


def _assemble_Z(results):
    Z = np.zeros(B, dtype=np.float64)
    for c in range(NCORES):
        r0 = c * RPC
        zr = np.asarray(results[c]["zrow"], np.float64)  # [128, 8]
        Z[r0 : r0 + RPC] += zr.T.reshape(-1)
        zxv = np.asarray(results[c]["zx"], np.float64)  # [128, 4]
        xoff = 0 if c < 4 else 512
        Z[r0 + xoff : r0 + xoff + 512] += zxv.T.reshape(-1)
        zc = np.asarray(results[c]["zcol"], np.float64).reshape(-1)  # [9*512]
        # units 0..8 -> rotated col ranges
        unit_cols = [512, 4096, 1024, 1536, 2048, 2560, 3072, 3584, None]
        for unit in range(8):
            j0 = unit_cols[unit]
            gcols = (r0 + j0 + np.arange(512)) % B
            Z[gcols] += zc[unit * 512 : (unit + 1) * 512]
        # unit 8: X cols
        xc0 = ((2 * c + 8) * 512) % B if c < 4 else ((2 * c - 7) * 512) % B
        Z[xc0 : xc0 + 512] += zc[8 * 512 : 9 * 512]
    return Z


def _device_Z(embeddings_f32: np.ndarray):
    run = _make_runner()
    results = run(_make_in_maps(embeddings_f32))
    Z = _assemble_Z(results)
    # subtract the unmasked diagonal: exp(||e_i||^2_fp8 * ISCALE)
    et = _quantize(embeddings_f32).astype(np.float64)  # [D, B]
    s_ii = np.einsum("di,di->i", et, et)
    Z -= np.exp(s_ii * ISCALE)
    return Z


def kernel(embeddings: np.ndarray, labels: np.ndarray) -> np.ndarray:
    E = np.asarray(embeddings, dtype=np.float32)
    labels = np.asarray(labels)

    Z = _device_Z(E)

    # Host epilogue in float64 (O(B*D) work).
    Ef = E.astype(np.float64)
    lse = np.log(Z)

    nclass = int(labels.max()) + 1
    counts = np.bincount(labels, minlength=nclass)
    num_pos = counts[labels] - 1
    G = np.zeros((nclass, D), dtype=np.float64)
    np.add.at(G, labels, Ef)
    sum_pos = (
        np.einsum("ij,ij->i", Ef, G[labels]) - np.einsum("ij,ij->i", Ef, Ef)
    ) / TAU
    mean_pos = sum_pos / np.maximum(num_pos, 1)
    has_pos = num_pos > 0
    loss_i = lse - mean_pos
    loss = np.sum(np.where(has_pos, loss_i, 0.0)) / max(int(has_pos.sum()), 1)
    return np.float32(loss)


# revision 23
# speedup vs baseline: 1.0947x; 1.0434x over previous
"""GroupSupConLoss on 8 Trainium2 NeuronCores — symmetric (triangle) version.

Device computes Z[i] = sum_{j != i} exp(sim[i,j]) exploiting sim's symmetry:
each exp'd element feeds TWO Z entries (its row via ACT accum row-sums, its
column via fp8 DoubleRow ones-matmul column-sums), so each core computes only
~9/16 of the columns the naive row-block split needs.

Coverage (rotated coords; col j of core c is global (c*1024+j) % 8192).
Row tiles rt 0-7 are the core's own 1024 rows; per-chunk column extents:
  C0 Da   : rts 0-3 x cols    0:1024  rowsums; colsum on cols 512:1024
  C1 DbB3 : rts 4-7 x cols  512:1024 | 4096:4608   rowsums; colsum on B3 part
  C2 B1   : rts 0-7 x cols 1024:2560  rowsums + colsums
  C3 B2   : rts 0-7 x cols 2560:4096  rowsums + colsums
  C4 X    : distance-8 512x512 quadrant (per-core stationary wxpl / moving
            exin): cores 0-3 rows 0:512 x global strip 2c+8;
                   cores 4-7 rows 512:1024 x global strip 2c-7
The self-diagonal exp(sim[i,i]) is NOT masked on device; the host subtracts
exp(||e_i||^2_fp8 / tau) computed exactly from the quantized embeddings.
Host maps col-sum vectors back to global rows, assembles Z, then does the
O(B*D) positives math in float64.

GEMM is fp8e4 (embeddings scaled by 32), DoubleRow perf mode (K=256 per
instruction), with bank-inner stationary reuse to amortize PE weight loads
(which are not hidden on real TRN2 and dominate per-matmul cost).
"""

import numpy as np
import ml_dtypes

import concourse.bacc as bacc
import concourse.mybir as mybir
from concourse.tile import TileContext

B = 8192           # batch
D = 1024           # embed dim
NCORES = 8
RPC = B // NCORES  # rows per core = 1024
NK = D // 128      # 8 contraction chunks
NKP = NK // 2      # 4 DoubleRow k-pairs
NRT = RPC // 128   # 8 row tiles per core
WCOLS = 1024       # resident own-block cols
TAU = 0.1
FP8_SCALE = 32.0
ISCALE = 1.0 / (TAU * FP8_SCALE * FP8_SCALE)

DRMODE = mybir.MatmulPerfMode.DoubleRow

_NC_CACHE = {}


def _dedup_ldweights(nc):
    """Remove back-to-back InstLdweights that reload the identical stationary
    operand (same tensor/offset/access-pattern/perf-mode). The PE weight array
    persists across matmuls, and walrus pairs each non-self-loading matmul
    with the most recent ldweights, so consecutive duplicates are pure
    overhead (~107ns each for fp8 DoubleRow loads). Loads carrying semaphore
    waits or updates are kept (bacc moved matmul waits onto them)."""
    removed = 0
    for fn in nc.m.functions:
        for bb in fn.blocks:
            il = bb.instructions
            last_key = None
            i = 0
            while i < len(il):
                inst = il[i]
                if type(inst).__name__ == "InstLdweights":
                    ap = inst.ins[0]
                    key = (
                        getattr(ap, "offset", None),
                        str(getattr(ap, "ap", None)),
                        str(getattr(ap, "memref", None)),
                        str(getattr(inst, "perf_mode", None)),
                    )
                    si = inst.sync_info
                    clean = si is None or (not si.on_wait and not si.on_update)
                    if key == last_key and clean:
                        del il[i]
                        removed += 1
                        continue
                    last_key = key
                i += 1
    return removed


def _build_nc(
    reps: int = 1,
    fori: bool = False,
    no_csum: bool = False,
    ex_bufs: int = 3,
    x_early: bool = True,
    dedup_lw: bool = True,
):
    nc = bacc.Bacc(None, target_bir_lowering=False)
    f8 = mybir.dt.float8e4
    # rotated E^T cols 0:4608 (all this core reads)
    etrot = nc.declare_dram_parameter("etrot", [D, 4608], f8, isOutput=False)
    # X-quadrant stationary (own 512-row half) and moving operand
    wxpl = nc.declare_dram_parameter("wxpl", [D, 512], f8, isOutput=False)
    exin = nc.declare_dram_parameter("exin", [D, 512], f8, isOutput=False)
    onesd = nc.declare_dram_parameter("onesd", [128, 256], f8, isOutput=False)
    zrow = nc.declare_dram_parameter("zrow", [128, NRT], mybir.dt.float32, isOutput=True)
    zx = nc.declare_dram_parameter("zx", [128, 4], mybir.dt.float32, isOutput=True)
    zcol = nc.declare_dram_parameter("zcol", [1, 9 * 512], mybir.dt.float32, isOutput=True)

    et3 = etrot.rearrange("(nk p) c -> p nk c", p=128)
    wx3 = wxpl.rearrange("(nk p) c -> p nk c", p=128)
    ex3 = exin.rearrange("(nk p) c -> p nk c", p=128)

    with TileContext(nc) as tc:
        with (
            tc.tile_pool(name="singles", bufs=1) as singles,
            tc.tile_pool(name="rhsp", bufs=2) as rhsp,
            tc.tile_pool(name="psump", bufs=2, space="PSUM") as psump,
            tc.tile_pool(name="cpsum", bufs=2, space="PSUM") as cpsum,
            tc.tile_pool(name="expp", bufs=ex_bufs) as expp,
        ):
            W = singles.tile([128, NK, WCOLS], f8, name="W")
            for k in range(NK):
                nc.sync.dma_start(out=W[:, k : k + 1, :], in_=et3[:, k : k + 1, 0:WCOLS])
            wx_sb = singles.tile([128, NK, 512], f8, name="wx_sb")
            nc.sync.dma_start(out=wx_sb[:, :, :], in_=wx3[:, :, :])
            exin_sb = singles.tile([128, NK, 512], f8, name="exin_sb")
            nc.sync.dma_start(out=exin_sb[:, :, :], in_=ex3[:, :, :])
            ones_sb = singles.tile([128, 2, 128], f8, name="ones_sb")
            nc.sync.dma_start(
                out=ones_sb[:, :, :],
                in_=onesd.rearrange("p (two m) -> p two m", two=2)[:, :, :],
            )

            acc = singles.tile([128, NRT, 3], mybir.dt.float32, name="acc")
            accx = singles.tile([128, 4], mybir.dt.float32, name="accx")
            zt = singles.tile([128, NRT], mybir.dt.float32, name="zt")
            zc_sb = singles.tile([1, 9 * 512], mybir.dt.float32, name="zc_sb")
            if no_csum:
                nc.vector.memset(zc_sb[:, :], 0.0)

            # chunk descriptors:
            # (name, rts, nunits, moving_fn, stationary_fn, slot, csum_units)
            # moving_fn(kp, u) -> AP [128, 2, 512]; csum_units: list of
            # (unit_idx_in_chunk, zcol_unit_idx)
            def _body(rep):
                # --- C0 "Da": rts 0-3, cols 0:1024 ---
                # --- C1 "DbB3": rts 4-7, cols 512:1024 | 4096:4608 ---
                rhs_b3 = rhsp.tile([128, NK, 512], f8, name=f"rhsb3_{rep}", tag="rhs3")
                nc.sync.dma_start(out=rhs_b3[:, :, :], in_=et3[:, :, 4096:4608])
                # --- C2/C3 B-groups 1536 cols each ---
                rhs_b1 = rhsp.tile([128, NK, 1536], f8, name=f"rhsb1_{rep}", tag="rhs")
                nc.sync.dma_start(out=rhs_b1[:, :, :], in_=et3[:, :, 1024:2560])
                rhs_b2 = rhsp.tile([128, NK, 1536], f8, name=f"rhsb2_{rep}", tag="rhs")
                nc.sync.dma_start(out=rhs_b2[:, :, :], in_=et3[:, :, 2560:4096])

                chunks = [
                    # name, rt range, cols, moving aps per (kp, unit), slot,
                    # csum: list of (unit, zcol_slot)
                    ("Da", range(0, 4), 1024,
                     lambda kp, u: W[:, 2 * kp : 2 * kp + 2, u * 512 : u * 512 + 512],
                     None, 0, [(1, 0)]),
                    ("DbB3", range(4, 8), 1024,
                     lambda kp, u: (
                         W[:, 2 * kp : 2 * kp + 2, 512:1024] if u == 0
                         else rhs_b3[:, 2 * kp : 2 * kp + 2, :]
                     ),
                     None, 0, [(1, 1)]),
                    ("B1", range(0, 8), 1536,
                     lambda kp, u: rhs_b1[:, 2 * kp : 2 * kp + 2, u * 512 : u * 512 + 512],
                     None, 1, [(0, 2), (1, 3), (2, 4)]),
                    ("B2", range(0, 8), 1536,
                     lambda kp, u: rhs_b2[:, 2 * kp : 2 * kp + 2, u * 512 : u * 512 + 512],
                     None, 2, [(0, 5), (1, 6), (2, 7)]),
                    ("X", range(0, 4), 512,
                     lambda kp, u: exin_sb[:, 2 * kp : 2 * kp + 2, :],
                     wx_sb, None, [(0, 8)]),
                ]
                if x_early:
                    chunks = [chunks[0], chunks[4], chunks[1], chunks[2], chunks[3]]

                def emit_csums(name, csums, ex3d, nrts):
                    # column sums for the marked units: fp8 DoubleRow
                    # ones-matmuls over rt pairs
                    for u, zslot in (() if no_csum else csums):
                        pscol = cpsum.tile(
                            [128, 512], mybir.dt.float32,
                            name=f"pscol_{rep}_{name}_{u}", tag="pscol",
                        )
                        npr = nrts // 2
                        for pr in range(npr):
                            nc.tensor.matmul(
                                pscol,
                                ones_sb[:, :, :],
                                ex3d[:, 2 * pr : 2 * pr + 2, u * 512 : (u + 1) * 512],
                                start=(pr == 0),
                                stop=(pr == npr - 1),
                                perf_mode=DRMODE,
                                skip_group_check=True,
                            )
                        nc.vector.tensor_copy(
                            zc_sb[:, zslot * 512 : (zslot + 1) * 512], pscol[0:1, :]
                        )

                # Software pipelining: each chunk's column-sum burst depends
                # on the chunk's LAST exp tile, so it is emitted after the
                # NEXT chunk's first row tile to keep the PE stream busy
                # while that ACT drains.
                pending = None
                for name, rts, cols, moving, statsrc, slot, csums in chunks:
                    nunits = cols // 512
                    nrts = len(list(rts))
                    ex3d = expp.tile(
                        [128, nrts, cols], f8, name=f"ex_{rep}_{name}", tag="ex"
                    )
                    for ri, rt in enumerate(rts):
                        ps = psump.tile(
                            [128, cols], mybir.dt.float32,
                            name=f"ps_{rep}_{name}_{rt}", tag="ps",
                        )
                        for kp in range(NKP):
                            if statsrc is None:
                                lhsT = W[:, 2 * kp : 2 * kp + 2, rt * 128 : (rt + 1) * 128]
                            else:
                                lhsT = statsrc[:, 2 * kp : 2 * kp + 2, ri * 128 : (ri + 1) * 128]
                            for u in range(nunits):
                                nc.tensor.matmul(
                                    ps[:, u * 512 : (u + 1) * 512],
                                    lhsT,
                                    moving(kp, u),
                                    start=(kp == 0),
                                    stop=(kp == NKP - 1),
                                    perf_mode=DRMODE,
                                    skip_group_check=True,
                                )
                        if name == "X":
                            acc_slot = accx[:, ri : ri + 1]
                        else:
                            acc_slot = acc[:, rt, slot : slot + 1]
                        nc.scalar.activation(
                            out=ex3d[:, ri, :],
                            in_=ps,
                            func=mybir.ActivationFunctionType.Exp,
                            scale=ISCALE,
                            accum_out=acc_slot,
                        )
                        if ri == 0 and pending is not None:
                            emit_csums(*pending)
                            pending = None
                    pending = (name, csums, ex3d, nrts)
                emit_csums(*pending)

            if fori and reps > 1:
                with tc.For_i(0, reps):
                    _body(0)
            else:
                for rep in range(reps):
                    _body(rep)

            nc.vector.reduce_sum(zt[:, :], acc[:, :, :], axis=mybir.AxisListType.X)
            nc.sync.dma_start(out=zrow[:, :], in_=zt)
            nc.sync.dma_start(out=zx[:, :], in_=accx)
            nc.sync.dma_start(out=zcol[:, :], in_=zc_sb)
    nc.finalize()
    if dedup_lw:
        _dedup_ldweights(nc)
    return nc


def _get_nc():
    if "nc" not in _NC_CACHE:
        _NC_CACHE["nc"] = _build_nc()
    return _NC_CACHE["nc"]


def _make_runner(nc=None, key="runner"):
    """Build a cached jitted SPMD executor for the bass program."""
    if key in _NC_CACHE:
        return _NC_CACHE[key]

    import jax
    import concourse.mybir as mybir_
    from concourse import bass2jax
    from concourse.bass2jax import _bass_exec_p, partition_id_tensor
    from jax.sharding import Mesh, PartitionSpec
    from jax.experimental.shard_map import shard_map

    if nc is None:
        nc = _get_nc()
    bass2jax.install_neuronx_cc_hook()

    partition_name = nc.partition_id_tensor.name if nc.partition_id_tensor else None
    in_names, out_names, out_avals, zero_outs = [], [], [], []
    for alloc in nc.m.functions[0].allocations:
        if not isinstance(alloc, mybir_.MemoryLocationSet):
            continue
        name = alloc.memorylocations[0].name
        if alloc.kind == "ExternalInput":
            if name != partition_name:
                in_names.append(name)
        elif alloc.kind == "ExternalOutput":
            shape = tuple(alloc.tensor_shape)
            dtype = mybir_.dt.np(alloc.dtype)
            out_names.append(name)
            out_avals.append(jax.core.ShapedArray(shape, dtype))
            zero_outs.append(np.zeros(shape, dtype))
    n_params = len(in_names)
    all_in_names = list(in_names) + list(out_names)
    if partition_name is not None:
        all_in_names.append(partition_name)
    donate = tuple(range(n_params, n_params + len(out_avals)))

    def _bodyfn(*args):
        operands = list(args)
        if partition_name is not None:
            operands.append(partition_id_tensor())
        outs = _bass_exec_p.bind(
            *operands,
            out_avals=tuple(out_avals),
            in_names=tuple(all_in_names),
            out_names=tuple(out_names),
            lowering_input_output_aliases=(),
            sim_require_finite=True,
            sim_require_nnan=True,
            nc=nc,
        )
        return tuple(outs)

    devices = jax.devices()[:NCORES]
    mesh = Mesh(np.asarray(devices), ("core",))
    spec = PartitionSpec("core")
    sharded = jax.jit(
        shard_map(
            _bodyfn,
            mesh=mesh,
            in_specs=(spec,) * (n_params + len(out_avals)),
            out_specs=(spec,) * len(out_names),
            check_rep=False,
        ),
        donate_argnums=donate,
        keep_unused=True,
    )

    def run(in_maps, staged=None):
        if staged is None:
            concat_in = [
                np.concatenate([np.asarray(m[name]) for m in in_maps], axis=0)
                for name in in_names
            ]
        else:
            concat_in = staged
        concat_zeros = [
            np.zeros((NCORES * z.shape[0], *z.shape[1:]), z.dtype) for z in zero_outs
        ]
        out_arrs = sharded(*concat_in, *concat_zeros)
        return [
            {
                name: np.asarray(out_arrs[i]).reshape(NCORES, *out_avals[i].shape)[c]
                for i, name in enumerate(out_names)
            }
            for c in range(NCORES)
        ]

    run.in_names = in_names
    run.mesh = mesh
    run.spec = spec
    run.sharded = sharded
    run.zero_outs = zero_outs
    _NC_CACHE[key] = run
    return run


def _quantize(embeddings_f32: np.ndarray):
    """Returns (et8, e8f): [D, B] fp8 of E^T*scale, and [B, D] float32 of the
    dequantized values (for the host diag correction)."""
    et8 = np.ascontiguousarray(embeddings_f32.T * FP8_SCALE).astype(
        ml_dtypes.float8_e4m3
    )
    return et8


def _make_in_maps(embeddings_f32: np.ndarray):
    et = _quantize(embeddings_f32)  # [D, B] fp8
    ones2 = np.ones((128, 256), dtype=ml_dtypes.float8_e4m3)

    in_maps = []
    for c in range(NCORES):
        etrot_full = np.roll(et, -c * RPC, axis=1)
        etrot = np.ascontiguousarray(etrot_full[:, :4608])
        if c < 4:
            exin = etrot_full[:, 4096:4608]   # global strip 2c+8
            wxpl = etrot_full[:, 0:512]       # own rows 0:512
        else:
            exin = etrot_full[:, 4608:5120]   # global strip 2c-7
            wxpl = etrot_full[:, 512:1024]    # own rows 512:1024
        in_maps.append(
            {
                "etrot": etrot,
                "wxpl": np.ascontiguousarray(wxpl),
                "exin": np.ascontiguousarray(exin),
                "onesd": ones2,
            }
        )
    return in_maps


def _assemble_Z(results):
    Z = np.zeros(B, dtype=np.float64)
    for c in range(NCORES):
        r0 = c * RPC
        zr = np.asarray(results[c]["zrow"], np.float64)  # [128, 8]
        Z[r0 : r0 + RPC] += zr.T.reshape(-1)
        zxv = np.asarray(results[c]["zx"], np.float64)  # [128, 4]
        xoff = 0 if c < 4 else 512
        Z[r0 + xoff : r0 + xoff + 512] += zxv.T.reshape(-1)
        zc = np.asarray(results[c]["zcol"], np.float64).reshape(-1)  # [9*512]
        # units 0..8 -> rotated col ranges
        unit_cols = [512, 4096, 1024, 1536, 2048, 2560, 3072, 3584, None]
        for unit in range(8):
            j0 = unit_cols[unit]
            gcols = (r0 + j0 + np.arange(512)) % B
            Z[gcols] += zc[unit * 512 : (unit + 1) * 512]
        # unit 8: X cols
        xc0 = ((2 * c + 8) * 512) % B if c < 4 else ((2 * c - 7) * 512) % B
        Z[xc0 : xc0 + 512] += zc[8 * 512 : 9 * 512]
    return Z


def _device_Z(embeddings_f32: np.ndarray):
    run = _make_runner()
    results = run(_make_in_maps(embeddings_f32))
    Z = _assemble_Z(results)
    # subtract the unmasked diagonal: exp(||e_i||^2_fp8 * ISCALE)
    et = _quantize(embeddings_f32).astype(np.float64)  # [D, B]
    s_ii = np.einsum("di,di->i", et, et)
    Z -= np.exp(s_ii * ISCALE)
    return Z


def kernel(embeddings: np.ndarray, labels: np.ndarray) -> np.ndarray:
    E = np.asarray(embeddings, dtype=np.float32)
    labels = np.asarray(labels)

    Z = _device_Z(E)

    # Host epilogue in float64 (O(B*D) work).
    Ef = E.astype(np.float64)
    lse = np.log(Z)

    nclass = int(labels.max()) + 1
    counts = np.bincount(labels, minlength=nclass)
    num_pos = counts[labels] - 1
    G = np.zeros((nclass, D), dtype=np.float64)
    np.add.at(G, labels, Ef)
    sum_pos = (
        np.einsum("ij,ij->i", Ef, G[labels]) - np.einsum("ij,ij->i", Ef, Ef)
    ) / TAU
    mean_pos = sum_pos / np.maximum(num_pos, 1)
    has_pos = num_pos > 0
    loss_i = lse - mean_pos
    loss = np.sum(np.where(has_pos, loss_i, 0.0)) / max(int(has_pos.sum()), 1)
    return np.float32(loss)
